# revision 4
# baseline (speedup 1.0000x reference)
"""GraphSAGE (2-layer, mean aggregation) on 8 Trainium2 NeuronCores.

Sharding: nodes split into 8 contiguous shards (12544 each, N padded
100000->100352). Edges partitioned by destination shard; within a shard,
sorted by dst and grouped into 98 blocks of 128 dst nodes, each padded to
a common chunk count (chunks of 128 edges).

Per block b, per chunk c (on the owning core):
  gather   M[e,:] = xg[src[e], :]            (indirect DMA, bf16 table)
  one-hot  P[e,d] = (eloc[e] == d)           (DVE is_equal vs iota)
  scatter  S^T += M^T @ P  (layer1, [feat,dst])  /  S += P^T @ Mz (layer2)
Then the dense branches: h1 = l2norm(relu([x@w1s+b1s, (S/deg)@w1n+b1n])),
z = h1@w2n (AllGather'd bf16 between layers), layer 2 symmetric, head fc.

Mean + bias via identity  (S + deg*b) * (1/max(deg,1)) == S/deg + b,
with deg*b added in PSUM by a K=1 matmul (skipped when biases are zero).
"""
import numpy as np
from ml_dtypes import bfloat16

import concourse.bass as bass
import concourse.bacc as bacc
import concourse.tile as tile
import concourse.mybir as mybir
from concourse.bass_utils import run_bass_kernel_spmd
from concourse.bass import IndirectOffsetOnAxis

P = 128
NCORES = 8
N = 100000
NPAD = 100352            # 8 * 12544
SH = NPAD // NCORES      # 12544
NBLK = SH // P           # 98
NFEAT = 128
NHID = 128
NCLS = 40

_cache = {}
_last_run = None


def _build(nch, with_bias):
    nc = bacc.Bacc("TRN2", target_bir_lowering=False, debug=False,
                   num_devices=NCORES)
    dt = mybir.dt
    f32, bf16, i32 = dt.float32, dt.bfloat16, dt.int32

    xg_d = nc.dram_tensor("xg", [NPAD, P], bf16, kind="ExternalInput")
    xsT_d = nc.dram_tensor("xsT", [P, SH], f32, kind="ExternalInput")
    esrc_d = nc.dram_tensor("esrc", [P, NBLK * nch], i32, kind="ExternalInput")
    eloc_d = nc.dram_tensor("eloc", [P, NBLK * nch], f32, kind="ExternalInput")
    invd_d = nc.dram_tensor("invd", [P, NBLK], f32, kind="ExternalInput")
    iota_d = nc.dram_tensor("iota", [P, P], bf16, kind="ExternalInput")
    ident_d = nc.dram_tensor("ident", [P, P], f32, kind="ExternalInput")
    w_d = {}
    for nm in ("w1s", "w1n", "w2sa", "w2sb", "w2na", "w2nb"):
        w_d[nm] = nc.dram_tensor(nm, [P, P], f32, kind="ExternalInput")
    w_d["wfca"] = nc.dram_tensor("wfca", [P, NCLS], f32, kind="ExternalInput")
    w_d["wfcb"] = nc.dram_tensor("wfcb", [P, NCLS], f32, kind="ExternalInput")
    if with_bias:
        degc_d = nc.dram_tensor("degc", [1, SH], f32, kind="ExternalInput")
        bias_d = {}
        for nm, w in (("b1s", P), ("b1n", P), ("b2s", P), ("b2n", P),
                      ("bfc", NCLS)):
            bias_d[nm] = nc.dram_tensor(nm, [1, w], f32, kind="ExternalInput")
    out_d = nc.dram_tensor("out", [SH, NCLS], f32, kind="ExternalOutput")

    with tile.TileContext(nc) as tc:
        with (
            tc.tile_pool(name="const", bufs=1) as cp,
            tc.tile_pool(name="big", bufs=1) as bigp,
            tc.tile_pool(name="msg", bufs=4) as mp,
            tc.tile_pool(name="oh", bufs=4) as ohp,
            tc.tile_pool(name="work", bufs=3) as wp,
            tc.tile_pool(name="ps_agg", bufs=2, space="PSUM") as ps_agg,
            tc.tile_pool(name="ps_mm", bufs=2, space="PSUM") as ps_mm,
            tc.tile_pool(name="dram", bufs=1, space="DRAM") as dp,
        ):
            # ---- constants into SBUF ----
            esrc_sb = cp.tile([P, NBLK * nch], i32)
            nc.sync.dma_start(out=esrc_sb[:], in_=esrc_d[:, :])
            eloc_sb = cp.tile([P, NBLK * nch], f32)
            nc.sync.dma_start(out=eloc_sb[:], in_=eloc_d[:, :])
            invd_sb = cp.tile([P, NBLK], f32)
            nc.sync.dma_start(out=invd_sb[:], in_=invd_d[:, :])
            iota_sb = cp.tile([P, P], bf16)
            nc.sync.dma_start(out=iota_sb[:], in_=iota_d[:, :])
            ident_sb = cp.tile([P, P], f32)
            nc.sync.dma_start(out=ident_sb[:], in_=ident_d[:, :])
            w_sb = {}
            for nm, d in w_d.items():
                w_sb[nm] = cp.tile([P, P if not nm.startswith("wfc") else NCLS],
                                   f32, name=f"w_{nm}")
                nc.sync.dma_start(out=w_sb[nm][:], in_=d[:, :])
            if with_bias:
                degc_sb = cp.tile([1, SH], f32)
                nc.sync.dma_start(out=degc_sb[:], in_=degc_d[:, :])
                ones_sb = cp.tile([1, P], f32)
                nc.any.memset(ones_sb[:], 1.0)
                b_sb = {}
                for nm, d in bias_d.items():
                    wdt = NCLS if nm == "bfc" else P
                    b_sb[nm] = cp.tile([1, wdt], f32, name=f"b_{nm}")
                    nc.sync.dma_start(out=b_sb[nm][:], in_=d[:, :])

            h2a_all = bigp.tile([P, NBLK * P], f32)      # 6.4 MB
            z_all = bigp.tile([P, NBLK * P], bf16)       # 3.2 MB
            out_all = bigp.tile([P, NBLK * NCLS], f32)   # 2.0 MB

            z_loc = dp.tile([SH, P], bf16)
            z_full = dp.tile([NPAD, P], bf16)

            def aggregate(b, table_d, transposed):
                """Accumulate chunk matmuls for block b.
                transposed=True  -> psum [feat, dst] = sum M^T @ P  (layer 1)
                transposed=False -> psum [dst, feat] = sum P^T @ M  (layer 2)
                Returns the psum tile (accumulation group left OPEN: caller
                must issue the closing matmul with stop=True), plus a closer.
                """
                agg = ps_agg.tile([P, P], mybir.dt.float32, tag="agg",
                                  name=f"agg{b}")
                for c in range(nch):
                    col = b * nch + c
                    m = mp.tile([P, P], mybir.dt.bfloat16, tag="m", name=f"m{col}")
                    nc.gpsimd.indirect_dma_start(
                        out=m[:], out_offset=None, in_=table_d[:, :],
                        in_offset=IndirectOffsetOnAxis(
                            ap=esrc_sb[:, col:col + 1], axis=0),
                    )
                    oh = ohp.tile([P, P], mybir.dt.bfloat16, tag="oh",
                                  name=f"oh{col}")
                    nc.vector.tensor_scalar(
                        out=oh[:], in0=iota_sb[:],
                        scalar1=eloc_sb[:, col:col + 1], scalar2=None,
                        op0=mybir.AluOpType.is_equal)
                    lhsT, rhs = (m, oh) if transposed else (oh, m)
                    nc.tensor.matmul(out=agg[:], lhsT=lhsT[:], rhs=rhs[:],
                                     start=(c == 0), stop=(c == nch - 1))
                return agg

            def l2norm(h):
                """h: [P, 256] f32 sbuf tile, normalized in place."""
                sq = wp.tile([P, 2 * P], mybir.dt.float32, tag="sq", name="sq")
                n2 = wp.tile([P, 1], mybir.dt.float32, tag="n2", name="n2")
                nc.scalar.activation(out=sq[:], in_=h[:],
                                     func=mybir.ActivationFunctionType.Square,
                                     accum_out=n2[:])
                nr = wp.tile([P, 1], mybir.dt.float32, tag="nr", name="nr")
                nc.scalar.sqrt(nr[:], n2[:])
                nc.vector.tensor_scalar(out=nr[:], in0=nr[:], scalar1=1e-12,
                                        scalar2=None, op0=mybir.AluOpType.max)
                ri = wp.tile([P, 1], mybir.dt.float32, tag="ri", name="ri")
                nc.vector.reciprocal(ri[:], nr[:])
                nc.vector.tensor_scalar(out=h[:], in0=h[:], scalar1=ri[:, :1],
                                        scalar2=None, op0=mybir.AluOpType.mult)

            def transpose_pair(h):
                """h [P, 256] -> (haT, hbT) each [P, P] f32 sbuf."""
                outs = []
                for half in range(2):
                    tp = ps_mm.tile([P, P], mybir.dt.float32, tag="tp",
                                    name=f"tp{half}")
                    nc.tensor.transpose(out=tp[:],
                                        in_=h[:, half * P:(half + 1) * P],
                                        identity=ident_sb[:])
                    ht = wp.tile([P, P], mybir.dt.float32, tag=f"ht{half}",
                                 name=f"ht{half}")
                    nc.vector.tensor_copy(out=ht[:], in_=tp[:])
                    outs.append(ht)
                return outs

            # ================= phase A =================
            for b in range(NBLK):
                aggT = aggregate(b, xg_d, transposed=True)
                aggT_sb = wp.tile([P, P], mybir.dt.float32, tag="aggsb",
                                  name=f"aggsb{b}")
                nc.vector.tensor_copy(out=aggT_sb[:], in_=aggT[:])

                xsT_blk = wp.tile([P, P], mybir.dt.float32, tag="xsT",
                                  name=f"xsT{b}")
                nc.sync.dma_start(out=xsT_blk[:],
                                  in_=xsT_d[:, b * P:(b + 1) * P])

                h1 = wp.tile([P, 2 * P], mybir.dt.float32, tag="h1", name=f"h1_{b}")
                # self branch
                ps_a = ps_mm.tile([P, P], mybir.dt.float32, tag="mm", name=f"psa{b}")
                nc.tensor.matmul(out=ps_a[:], lhsT=xsT_blk[:], rhs=w_sb["w1s"][:],
                                 start=True, stop=not with_bias)
                if with_bias:
                    nc.tensor.matmul(out=ps_a[:], lhsT=ones_sb[:, :P],
                                     rhs=b_sb["b1s"][:], start=False, stop=True)
                nc.vector.tensor_scalar(out=h1[:, :P], in0=ps_a[:], scalar1=0.0,
                                        scalar2=None, op0=mybir.AluOpType.max)
                # neighbor branch: (S@w1n + deg*b1n) * invd, relu
                ps_b = ps_mm.tile([P, P], mybir.dt.float32, tag="mm", name=f"psb{b}")
                nc.tensor.matmul(out=ps_b[:], lhsT=aggT_sb[:], rhs=w_sb["w1n"][:],
                                 start=True, stop=not with_bias)
                if with_bias:
                    nc.tensor.matmul(out=ps_b[:], lhsT=degc_sb[:, b * P:(b + 1) * P],
                                     rhs=b_sb["b1n"][:], start=False, stop=True)
                nc.vector.tensor_scalar(out=h1[:, P:], in0=ps_b[:],
                                        scalar1=invd_sb[:, b:b + 1], scalar2=0.0,
                                        op0=mybir.AluOpType.mult,
                                        op1=mybir.AluOpType.max)
                l2norm(h1)
                h1aT, h1bT = transpose_pair(h1)
                # z = h1 @ w2n  -> bf16 into z_all
                ps_z = ps_mm.tile([P, P], mybir.dt.float32, tag="mm", name=f"psz{b}")
                nc.tensor.matmul(out=ps_z[:], lhsT=h1aT[:], rhs=w_sb["w2na"][:],
                                 start=True, stop=False)
                nc.tensor.matmul(out=ps_z[:], lhsT=h1bT[:], rhs=w_sb["w2nb"][:],
                                 start=False, stop=True)
                nc.vector.tensor_copy(out=z_all[:, b * P:(b + 1) * P], in_=ps_z[:])
                # h2a_raw = h1 @ w2s (+ b2s)
                ps_h = ps_mm.tile([P, P], mybir.dt.float32, tag="mm", name=f"psh{b}")
                nc.tensor.matmul(out=ps_h[:], lhsT=h1aT[:], rhs=w_sb["w2sa"][:],
                                 start=True, stop=False)
                nc.tensor.matmul(out=ps_h[:], lhsT=h1bT[:], rhs=w_sb["w2sb"][:],
                                 start=False, stop=not with_bias)
                if with_bias:
                    nc.tensor.matmul(out=ps_h[:], lhsT=ones_sb[:, :P],
                                     rhs=b_sb["b2s"][:], start=False, stop=True)
                nc.vector.tensor_copy(out=h2a_all[:, b * P:(b + 1) * P], in_=ps_h[:])

            # z -> DRAM, AllGather
            nc.sync.dma_start(
                out=z_loc[:].rearrange("(b p) c -> p b c", p=P),
                in_=z_all[:].rearrange("p (b c) -> p b c", c=P))
            nc.gpsimd.collective_compute(
                "AllGather", mybir.AluOpType.bypass,
                replica_groups=[list(range(NCORES))],
                ins=[z_loc.opt()], outs=[z_full.opt()])

            # ================= phase C =================
            for b in range(NBLK):
                agg2 = aggregate(b, z_full, transposed=False)
                h2 = wp.tile([P, 2 * P], mybir.dt.float32, tag="h2", name=f"h2_{b}")
                nc.vector.tensor_scalar(out=h2[:, :P],
                                        in0=h2a_all[:, b * P:(b + 1) * P],
                                        scalar1=0.0, scalar2=None,
                                        op0=mybir.AluOpType.max)
                if with_bias:
                    # reopen accumulation handled inline above (stop on last chunk
                    # was already emitted); add deg*b2n via separate psum read is
                    # not possible -> fold bias before scaling using DVE instead.
                    tmp = wp.tile([P, P], mybir.dt.float32, tag="tmpb", name=f"tb{b}")
                    nc.vector.tensor_scalar(out=tmp[:], in0=agg2[:],
                                            scalar1=invd_sb[:, b:b + 1],
                                            scalar2=None,
                                            op0=mybir.AluOpType.mult)
                    nc.vector.tensor_tensor(
                        out=h2[:, P:], in0=tmp[:],
                        in1=b_sb["b2n"][:].to_broadcast([P, P]),
                        op=mybir.AluOpType.add)
                    nc.vector.tensor_scalar(out=h2[:, P:], in0=h2[:, P:],
                                            scalar1=0.0, scalar2=None,
                                            op0=mybir.AluOpType.max)
                else:
                    nc.vector.tensor_scalar(out=h2[:, P:], in0=agg2[:],
                                            scalar1=invd_sb[:, b:b + 1],
                                            scalar2=0.0,
                                            op0=mybir.AluOpType.mult,
                                            op1=mybir.AluOpType.max)
                l2norm(h2)
                h2aT, h2bT = transpose_pair(h2)
                ps_o = ps_mm.tile([P, NCLS], mybir.dt.float32, tag="mm",
                                  name=f"pso{b}")
                nc.tensor.matmul(out=ps_o[:], lhsT=h2aT[:], rhs=w_sb["wfca"][:],
                                 start=True, stop=False)
                nc.tensor.matmul(out=ps_o[:], lhsT=h2bT[:], rhs=w_sb["wfcb"][:],
                                 start=False, stop=not with_bias)
                if with_bias:
                    nc.tensor.matmul(out=ps_o[:], lhsT=ones_sb[:, :P],
                                     rhs=b_sb["bfc"][:], start=False, stop=True)
                nc.vector.tensor_copy(out=out_all[:, b * NCLS:(b + 1) * NCLS],
                                      in_=ps_o[:])

            nc.sync.dma_start(
                out=out_d[:, :].rearrange("(b p) c -> p b c", p=P),
                in_=out_all[:].rearrange("p (b c) -> p b c", c=NCLS))

    nc.compile()
    return nc


def kernel(x, src, dst, w1s, b1s, w1n, b1n, w2s, b2s, w2n, b2n, wfc, bfc):
    x = np.asarray(x, np.float32)
    src = np.asarray(src, np.int32)
    dst = np.asarray(dst, np.int32)

    x_pad = np.zeros((NPAD, NFEAT), np.float32)
    x_pad[:N] = x
    xg = x_pad.astype(bfloat16)

    order = np.argsort(dst, kind="stable")
    ds, ss = dst[order], src[order]
    bounds = np.searchsorted(ds, np.arange(0, NPAD + 1, P))
    cnts = np.diff(bounds)                       # edges per 128-dst block
    nch = max(1, int(-(-cnts.max() // P)))       # chunks per block

    deg = np.bincount(dst, minlength=NPAD).astype(np.float32)
    invdeg = 1.0 / np.maximum(deg, 1.0)

    with_bias = any(np.any(np.asarray(b) != 0) for b in (b1s, b1n, b2s, b2n, bfc))

    nblk_g = NPAD // P
    esrc_g = np.zeros((nblk_g, nch * P), np.int32)
    eloc_g = np.full((nblk_g, nch * P), -1.0, np.float32)
    for g in range(nblk_g):
        s, e = bounds[g], bounds[g + 1]
        m = e - s
        esrc_g[g, :m] = ss[s:e]
        eloc_g[g, :m] = (ds[s:e] % P).astype(np.float32)
    # [g, c*P + p] -> [p, g*nch + c]
    esrc_pc = esrc_g.reshape(nblk_g, nch, P).transpose(2, 0, 1)
    eloc_pc = eloc_g.reshape(nblk_g, nch, P).transpose(2, 0, 1)

    iota_np = np.tile(np.arange(P, dtype=np.float32), (P, 1)).astype(bfloat16)
    ident_np = np.eye(P, dtype=np.float32)

    key = (nch, with_bias)
    if key not in _cache:
        _cache[key] = _build(nch, with_bias)
    nc = _cache[key]

    in_maps = []
    for k in range(NCORES):
        gs, ge = k * NBLK, (k + 1) * NBLK
        shard = slice(k * SH, (k + 1) * SH)
        m = {
            "xg": xg,
            "xsT": np.ascontiguousarray(x_pad[shard].T),
            "esrc": np.ascontiguousarray(
                esrc_pc[:, gs:ge].reshape(P, NBLK * nch)),
            "eloc": np.ascontiguousarray(
                eloc_pc[:, gs:ge].reshape(P, NBLK * nch)),
            "invd": np.ascontiguousarray(
                invdeg[shard].reshape(NBLK, P).T),
            "iota": iota_np,
            "ident": ident_np,
            "w1s": np.asarray(w1s, np.float32),
            "w1n": np.asarray(w1n, np.float32),
            "w2sa": np.asarray(w2s, np.float32)[:P],
            "w2sb": np.asarray(w2s, np.float32)[P:],
            "w2na": np.asarray(w2n, np.float32)[:P],
            "w2nb": np.asarray(w2n, np.float32)[P:],
            "wfca": np.asarray(wfc, np.float32)[:P],
            "wfcb": np.asarray(wfc, np.float32)[P:],
        }
        if with_bias:
            m["degc"] = np.maximum(deg[shard], 1.0).reshape(1, SH)
            m["b1s"] = np.asarray(b1s, np.float32).reshape(1, -1)
            m["b1n"] = np.asarray(b1n, np.float32).reshape(1, -1)
            m["b2s"] = np.asarray(b2s, np.float32).reshape(1, -1)
            m["b2n"] = np.asarray(b2n, np.float32).reshape(1, -1)
            m["bfc"] = np.asarray(bfc, np.float32).reshape(1, -1)
        in_maps.append(m)

    global _last_run
    _last_run = (nc, in_maps)
    res = run_bass_kernel_spmd(nc, in_maps, core_ids=list(range(NCORES)))
    out = np.concatenate([res.results[k]["out"] for k in range(NCORES)], axis=0)
    return out[:N].astype(np.float32)


# revision 5
# speedup vs baseline: 1.0631x; 1.0631x over previous
"""GraphSAGE (2-layer, mean aggregation) on 8 Trainium2 NeuronCores.

Sharding: nodes split into 8 contiguous shards (12544 each, N padded
100000->100352). Edges partitioned by destination shard; within a shard,
sorted by dst and grouped into 98 blocks of 128 dst nodes, each padded to
a common chunk count (chunks of 128 edges).

Per block b, per chunk c (on the owning core):
  gather   M[e,:] = xg[src[e], :]            (indirect DMA, bf16 table)
  one-hot  P[e,d] = (eloc[e] == d)           (DVE is_equal vs iota)
  scatter  S^T += M^T @ P  (layer1, [feat,dst])  /  S += P^T @ Mz (layer2)
Then the dense branches: h1 = l2norm(relu([x@w1s+b1s, (S/deg)@w1n+b1n])),
z = h1@w2n (AllGather'd bf16 between layers), layer 2 symmetric, head fc.

Mean + bias via identity  (S + deg*b) * (1/max(deg,1)) == S/deg + b,
with deg*b added in PSUM by a K=1 matmul (skipped when biases are zero).
"""
import numpy as np
from ml_dtypes import bfloat16

import concourse.bass as bass
import concourse.bacc as bacc
import concourse.tile as tile
import concourse.mybir as mybir
from concourse.bass_utils import run_bass_kernel_spmd
from concourse.bass import IndirectOffsetOnAxis

P = 128
NCORES = 8
N = 100000
NPAD = 100352            # 8 * 12544
SH = NPAD // NCORES      # 12544
NBLK = SH // P           # 98
NFEAT = 128
NHID = 128
NCLS = 40

_cache = {}
_last_run = None


def _build(nch_list, off_list, ncols, with_bias):
    nc = bacc.Bacc("TRN2", target_bir_lowering=False, debug=False,
                   num_devices=NCORES)
    dt = mybir.dt
    f32, bf16, i32 = dt.float32, dt.bfloat16, dt.int32

    xg_d = nc.dram_tensor("xg", [NPAD, P], bf16, kind="ExternalInput")
    xsT_d = nc.dram_tensor("xsT", [P, SH], f32, kind="ExternalInput")
    esrc_d = nc.dram_tensor("esrc", [P, ncols], i32, kind="ExternalInput")
    eloc_d = nc.dram_tensor("eloc", [P, ncols], f32, kind="ExternalInput")
    invd_d = nc.dram_tensor("invd", [P, NBLK], f32, kind="ExternalInput")
    iota_d = nc.dram_tensor("iota", [P, P], bf16, kind="ExternalInput")
    ident_d = nc.dram_tensor("ident", [P, P], f32, kind="ExternalInput")
    w_d = {}
    for nm in ("w1s", "w1n", "w2sa", "w2sb", "w2na", "w2nb"):
        w_d[nm] = nc.dram_tensor(nm, [P, P], f32, kind="ExternalInput")
    w_d["wfca"] = nc.dram_tensor("wfca", [P, NCLS], f32, kind="ExternalInput")
    w_d["wfcb"] = nc.dram_tensor("wfcb", [P, NCLS], f32, kind="ExternalInput")
    if with_bias:
        degc_d = nc.dram_tensor("degc", [1, SH], f32, kind="ExternalInput")
        bias_d = {}
        for nm, w in (("b1s", P), ("b1n", P), ("b2s", P), ("b2n", P),
                      ("bfc", NCLS)):
            bias_d[nm] = nc.dram_tensor(nm, [1, w], f32, kind="ExternalInput")
    out_d = nc.dram_tensor("out", [SH, NCLS], f32, kind="ExternalOutput")

    with tile.TileContext(nc) as tc:
        with (
            tc.tile_pool(name="const", bufs=1) as cp,
            tc.tile_pool(name="big", bufs=1) as bigp,
            tc.tile_pool(name="msg", bufs=4) as mp,
            tc.tile_pool(name="oh", bufs=4) as ohp,
            tc.tile_pool(name="work", bufs=3) as wp,
            tc.tile_pool(name="ps_agg", bufs=2, space="PSUM") as ps_agg,
            tc.tile_pool(name="ps_mm", bufs=2, space="PSUM") as ps_mm,
            tc.tile_pool(name="dram", bufs=1, space="DRAM") as dp,
        ):
            # ---- constants into SBUF ----
            esrc_sb = cp.tile([P, ncols], i32)
            nc.sync.dma_start(out=esrc_sb[:], in_=esrc_d[:, :])
            eloc_sb = cp.tile([P, ncols], f32)
            nc.sync.dma_start(out=eloc_sb[:], in_=eloc_d[:, :])
            invd_sb = cp.tile([P, NBLK], f32)
            nc.sync.dma_start(out=invd_sb[:], in_=invd_d[:, :])
            iota_sb = cp.tile([P, P], bf16)
            nc.sync.dma_start(out=iota_sb[:], in_=iota_d[:, :])
            ident_sb = cp.tile([P, P], f32)
            nc.sync.dma_start(out=ident_sb[:], in_=ident_d[:, :])
            w_sb = {}
            for nm, d in w_d.items():
                w_sb[nm] = cp.tile([P, P if not nm.startswith("wfc") else NCLS],
                                   f32, name=f"w_{nm}")
                nc.sync.dma_start(out=w_sb[nm][:], in_=d[:, :])
            if with_bias:
                degc_sb = cp.tile([1, SH], f32)
                nc.sync.dma_start(out=degc_sb[:], in_=degc_d[:, :])
                ones_sb = cp.tile([1, P], f32)
                nc.any.memset(ones_sb[:], 1.0)
                b_sb = {}
                for nm, d in bias_d.items():
                    wdt = NCLS if nm == "bfc" else P
                    b_sb[nm] = cp.tile([1, wdt], f32, name=f"b_{nm}")
                    nc.sync.dma_start(out=b_sb[nm][:], in_=d[:, :])

            h2a_all = bigp.tile([P, NBLK * P], f32)      # 6.4 MB
            z_all = bigp.tile([P, NBLK * P], bf16)       # 3.2 MB
            out_all = bigp.tile([P, NBLK * NCLS], f32)   # 2.0 MB

            z_loc = dp.tile([SH, P], bf16)
            z_full = dp.tile([NPAD, P], bf16)

            def aggregate(b, table_d, transposed):
                """Accumulate chunk matmuls for block b.
                transposed=True  -> psum [feat, dst] = sum M^T @ P  (layer 1)
                transposed=False -> psum [dst, feat] = sum P^T @ M  (layer 2)
                Returns the psum tile (accumulation group left OPEN: caller
                must issue the closing matmul with stop=True), plus a closer.
                """
                agg = ps_agg.tile([P, P], mybir.dt.float32, tag="agg",
                                  name=f"agg{b}")
                nch = nch_list[b]
                for c in range(nch):
                    col = off_list[b] + c
                    m = mp.tile([P, P], mybir.dt.bfloat16, tag="m", name=f"m{col}")
                    nc.gpsimd.indirect_dma_start(
                        out=m[:], out_offset=None, in_=table_d[:, :],
                        in_offset=IndirectOffsetOnAxis(
                            ap=esrc_sb[:, col:col + 1], axis=0),
                    )
                    oh = ohp.tile([P, P], mybir.dt.bfloat16, tag="oh",
                                  name=f"oh{col}")
                    nc.vector.tensor_scalar(
                        out=oh[:], in0=iota_sb[:],
                        scalar1=eloc_sb[:, col:col + 1], scalar2=None,
                        op0=mybir.AluOpType.is_equal)
                    lhsT, rhs = (m, oh) if transposed else (oh, m)
                    nc.tensor.matmul(out=agg[:], lhsT=lhsT[:], rhs=rhs[:],
                                     start=(c == 0), stop=(c == nch - 1))
                return agg

            def l2norm(h):
                """h: [P, 256] f32 sbuf tile, normalized in place."""
                sq = wp.tile([P, 2 * P], mybir.dt.float32, tag="sq", name="sq")
                n2 = wp.tile([P, 1], mybir.dt.float32, tag="n2", name="n2")
                nc.scalar.activation(out=sq[:], in_=h[:],
                                     func=mybir.ActivationFunctionType.Square,
                                     accum_out=n2[:])
                nr = wp.tile([P, 1], mybir.dt.float32, tag="nr", name="nr")
                nc.scalar.sqrt(nr[:], n2[:])
                nc.vector.tensor_scalar(out=nr[:], in0=nr[:], scalar1=1e-12,
                                        scalar2=None, op0=mybir.AluOpType.max)
                ri = wp.tile([P, 1], mybir.dt.float32, tag="ri", name="ri")
                nc.vector.reciprocal(ri[:], nr[:])
                nc.vector.tensor_scalar(out=h[:], in0=h[:], scalar1=ri[:, :1],
                                        scalar2=None, op0=mybir.AluOpType.mult)

            def transpose_pair(h):
                """h [P, 256] -> (haT, hbT) each [P, P] f32 sbuf."""
                outs = []
                for half in range(2):
                    tp = ps_mm.tile([P, P], mybir.dt.float32, tag="tp",
                                    name=f"tp{half}")
                    nc.tensor.transpose(out=tp[:],
                                        in_=h[:, half * P:(half + 1) * P],
                                        identity=ident_sb[:])
                    ht = wp.tile([P, P], mybir.dt.float32, tag=f"ht{half}",
                                 name=f"ht{half}")
                    nc.vector.tensor_copy(out=ht[:], in_=tp[:])
                    outs.append(ht)
                return outs

            # ================= phase A =================
            for b in range(NBLK):
                aggT = aggregate(b, xg_d, transposed=True)
                aggT_sb = wp.tile([P, P], mybir.dt.float32, tag="aggsb",
                                  name=f"aggsb{b}")
                nc.vector.tensor_copy(out=aggT_sb[:], in_=aggT[:])

                xsT_blk = wp.tile([P, P], mybir.dt.float32, tag="xsT",
                                  name=f"xsT{b}")
                nc.sync.dma_start(out=xsT_blk[:],
                                  in_=xsT_d[:, b * P:(b + 1) * P])

                h1 = wp.tile([P, 2 * P], mybir.dt.float32, tag="h1", name=f"h1_{b}")
                # self branch
                ps_a = ps_mm.tile([P, P], mybir.dt.float32, tag="mm", name=f"psa{b}")
                nc.tensor.matmul(out=ps_a[:], lhsT=xsT_blk[:], rhs=w_sb["w1s"][:],
                                 start=True, stop=not with_bias)
                if with_bias:
                    nc.tensor.matmul(out=ps_a[:], lhsT=ones_sb[:, :P],
                                     rhs=b_sb["b1s"][:], start=False, stop=True)
                nc.vector.tensor_scalar(out=h1[:, :P], in0=ps_a[:], scalar1=0.0,
                                        scalar2=None, op0=mybir.AluOpType.max)
                # neighbor branch: (S@w1n + deg*b1n) * invd, relu
                ps_b = ps_mm.tile([P, P], mybir.dt.float32, tag="mm", name=f"psb{b}")
                nc.tensor.matmul(out=ps_b[:], lhsT=aggT_sb[:], rhs=w_sb["w1n"][:],
                                 start=True, stop=not with_bias)
                if with_bias:
                    nc.tensor.matmul(out=ps_b[:], lhsT=degc_sb[:, b * P:(b + 1) * P],
                                     rhs=b_sb["b1n"][:], start=False, stop=True)
                nc.vector.tensor_scalar(out=h1[:, P:], in0=ps_b[:],
                                        scalar1=invd_sb[:, b:b + 1], scalar2=0.0,
                                        op0=mybir.AluOpType.mult,
                                        op1=mybir.AluOpType.max)
                l2norm(h1)
                h1aT, h1bT = transpose_pair(h1)
                # z = h1 @ w2n  -> bf16 into z_all
                ps_z = ps_mm.tile([P, P], mybir.dt.float32, tag="mm", name=f"psz{b}")
                nc.tensor.matmul(out=ps_z[:], lhsT=h1aT[:], rhs=w_sb["w2na"][:],
                                 start=True, stop=False)
                nc.tensor.matmul(out=ps_z[:], lhsT=h1bT[:], rhs=w_sb["w2nb"][:],
                                 start=False, stop=True)
                nc.vector.tensor_copy(out=z_all[:, b * P:(b + 1) * P], in_=ps_z[:])
                # h2a_raw = h1 @ w2s (+ b2s)
                ps_h = ps_mm.tile([P, P], mybir.dt.float32, tag="mm", name=f"psh{b}")
                nc.tensor.matmul(out=ps_h[:], lhsT=h1aT[:], rhs=w_sb["w2sa"][:],
                                 start=True, stop=False)
                nc.tensor.matmul(out=ps_h[:], lhsT=h1bT[:], rhs=w_sb["w2sb"][:],
                                 start=False, stop=not with_bias)
                if with_bias:
                    nc.tensor.matmul(out=ps_h[:], lhsT=ones_sb[:, :P],
                                     rhs=b_sb["b2s"][:], start=False, stop=True)
                nc.vector.tensor_copy(out=h2a_all[:, b * P:(b + 1) * P], in_=ps_h[:])

            # z -> DRAM, AllGather
            nc.sync.dma_start(
                out=z_loc[:].rearrange("(b p) c -> p b c", p=P),
                in_=z_all[:].rearrange("p (b c) -> p b c", c=P))
            nc.gpsimd.collective_compute(
                "AllGather", mybir.AluOpType.bypass,
                replica_groups=[list(range(NCORES))],
                ins=[z_loc.opt()], outs=[z_full.opt()])

            # ================= phase C =================
            for b in range(NBLK):
                agg2 = aggregate(b, z_full, transposed=False)
                h2 = wp.tile([P, 2 * P], mybir.dt.float32, tag="h2", name=f"h2_{b}")
                nc.vector.tensor_scalar(out=h2[:, :P],
                                        in0=h2a_all[:, b * P:(b + 1) * P],
                                        scalar1=0.0, scalar2=None,
                                        op0=mybir.AluOpType.max)
                if with_bias:
                    # reopen accumulation handled inline above (stop on last chunk
                    # was already emitted); add deg*b2n via separate psum read is
                    # not possible -> fold bias before scaling using DVE instead.
                    tmp = wp.tile([P, P], mybir.dt.float32, tag="tmpb", name=f"tb{b}")
                    nc.vector.tensor_scalar(out=tmp[:], in0=agg2[:],
                                            scalar1=invd_sb[:, b:b + 1],
                                            scalar2=None,
                                            op0=mybir.AluOpType.mult)
                    nc.vector.tensor_tensor(
                        out=h2[:, P:], in0=tmp[:],
                        in1=b_sb["b2n"][:].to_broadcast([P, P]),
                        op=mybir.AluOpType.add)
                    nc.vector.tensor_scalar(out=h2[:, P:], in0=h2[:, P:],
                                            scalar1=0.0, scalar2=None,
                                            op0=mybir.AluOpType.max)
                else:
                    nc.vector.tensor_scalar(out=h2[:, P:], in0=agg2[:],
                                            scalar1=invd_sb[:, b:b + 1],
                                            scalar2=0.0,
                                            op0=mybir.AluOpType.mult,
                                            op1=mybir.AluOpType.max)
                l2norm(h2)
                h2aT, h2bT = transpose_pair(h2)
                ps_o = ps_mm.tile([P, NCLS], mybir.dt.float32, tag="mm",
                                  name=f"pso{b}")
                nc.tensor.matmul(out=ps_o[:], lhsT=h2aT[:], rhs=w_sb["wfca"][:],
                                 start=True, stop=False)
                nc.tensor.matmul(out=ps_o[:], lhsT=h2bT[:], rhs=w_sb["wfcb"][:],
                                 start=False, stop=not with_bias)
                if with_bias:
                    nc.tensor.matmul(out=ps_o[:], lhsT=ones_sb[:, :P],
                                     rhs=b_sb["bfc"][:], start=False, stop=True)
                nc.vector.tensor_copy(out=out_all[:, b * NCLS:(b + 1) * NCLS],
                                      in_=ps_o[:])

            nc.sync.dma_start(
                out=out_d[:, :].rearrange("(b p) c -> p b c", p=P),
                in_=out_all[:].rearrange("p (b c) -> p b c", c=NCLS))

    nc.compile()
    return nc


def kernel(x, src, dst, w1s, b1s, w1n, b1n, w2s, b2s, w2n, b2n, wfc, bfc):
    x = np.asarray(x, np.float32)
    src = np.asarray(src, np.int32)
    dst = np.asarray(dst, np.int32)

    x_pad = np.zeros((NPAD, NFEAT), np.float32)
    x_pad[:N] = x
    xg = x_pad.astype(bfloat16)

    order = np.argsort(dst, kind="stable")
    ds, ss = dst[order], src[order]
    bounds = np.searchsorted(ds, np.arange(0, NPAD + 1, P))
    cnts = np.diff(bounds)                       # edges per 128-dst block
    # chunks per block index b: max over the 8 cores owning that index
    nch_list = [max(1, int(-(-int(cnts[k * NBLK + b]) // P)))
                for b in range(NBLK) for k in [0]]
    nch_list = [max(max(1, int(-(-int(cnts[k * NBLK + b]) // P)))
                    for k in range(NCORES)) for b in range(NBLK)]
    off_list = np.concatenate([[0], np.cumsum(nch_list)]).astype(int)
    ncols = int(off_list[-1])

    deg = np.bincount(dst, minlength=NPAD).astype(np.float32)
    invdeg = 1.0 / np.maximum(deg, 1.0)

    with_bias = any(np.any(np.asarray(b) != 0) for b in (b1s, b1n, b2s, b2n, bfc))

    esrc_pcs = np.zeros((NCORES, P, ncols), np.int32)
    eloc_pcs = np.full((NCORES, P, ncols), -1.0, np.float32)
    for k in range(NCORES):
        for b in range(NBLK):
            g = k * NBLK + b
            s0, e0 = bounds[g], bounds[g + 1]
            m = e0 - s0
            nb = nch_list[b]
            ebuf = np.zeros(nb * P, np.int32)
            lbuf = np.full(nb * P, -1.0, np.float32)
            ebuf[:m] = ss[s0:e0]
            lbuf[:m] = (ds[s0:e0] % P).astype(np.float32)
            o = off_list[b]
            esrc_pcs[k, :, o:o + nb] = ebuf.reshape(nb, P).T
            eloc_pcs[k, :, o:o + nb] = lbuf.reshape(nb, P).T

    iota_np = np.tile(np.arange(P, dtype=np.float32), (P, 1)).astype(bfloat16)
    ident_np = np.eye(P, dtype=np.float32)

    key = (tuple(nch_list), with_bias)
    if key not in _cache:
        _cache[key] = _build(nch_list, off_list, ncols, with_bias)
    nc = _cache[key]

    in_maps = []
    for k in range(NCORES):
        gs, ge = k * NBLK, (k + 1) * NBLK
        shard = slice(k * SH, (k + 1) * SH)
        m = {
            "xg": xg,
            "xsT": np.ascontiguousarray(x_pad[shard].T),
            "esrc": esrc_pcs[k],
            "eloc": eloc_pcs[k],
            "invd": np.ascontiguousarray(
                invdeg[shard].reshape(NBLK, P).T),
            "iota": iota_np,
            "ident": ident_np,
            "w1s": np.asarray(w1s, np.float32),
            "w1n": np.asarray(w1n, np.float32),
            "w2sa": np.asarray(w2s, np.float32)[:P],
            "w2sb": np.asarray(w2s, np.float32)[P:],
            "w2na": np.asarray(w2n, np.float32)[:P],
            "w2nb": np.asarray(w2n, np.float32)[P:],
            "wfca": np.asarray(wfc, np.float32)[:P],
            "wfcb": np.asarray(wfc, np.float32)[P:],
        }
        if with_bias:
            m["degc"] = np.maximum(deg[shard], 1.0).reshape(1, SH)
            m["b1s"] = np.asarray(b1s, np.float32).reshape(1, -1)
            m["b1n"] = np.asarray(b1n, np.float32).reshape(1, -1)
            m["b2s"] = np.asarray(b2s, np.float32).reshape(1, -1)
            m["b2n"] = np.asarray(b2n, np.float32).reshape(1, -1)
            m["bfc"] = np.asarray(bfc, np.float32).reshape(1, -1)
        in_maps.append(m)

    global _last_run
    _last_run = (nc, in_maps)
    res = run_bass_kernel_spmd(nc, in_maps, core_ids=list(range(NCORES)))
    out = np.concatenate([res.results[k]["out"] for k in range(NCORES)], axis=0)
    return out[:N].astype(np.float32)
